# revision 73
# baseline (speedup 1.0000x reference)
"""Causal self-attention (GPT-2 style) Bass kernel for Trainium2.

B=8, T=1024, C=768, NH=12, HD=64. Data-parallel over batch: each of the 8
NeuronCores computes one batch element end to end.

Per-core plan (bf16 matmuls, fp32 PSUM):
  - x arrives bf16 via SWDGE converting DMA; W_attn streams as fp32 column
    slabs on HWDGE (q cols first) with DVE casts, so qkv matmuls start ~8us
    in; W_proj converts on the SWDGE ring.
  - q^T,k^T in [head_dim, T] layout (pairs of heads in 128 partitions);
    v natural [T, HD] augmented with a ones column (row 64 of the AV psum
    then holds the softmax denominator).
  - Scores S^T per head, causally trimmed: each 128-key x 512-query tile
    splits into a 128-wide diagonal band (masked by one identity-matmul of
    a replicated lower-tri tile per packed band bank) plus the fully-valid
    suffix; fully-masked columns are never computed. Pieces pack exactly
    into three 1536-col PSUM groups per head (all matmul regions inside a
    single 512-col bank); exp (ScalarE) writes a packed bf16 slab.
  - AV = v_aug.T @ P^T accumulated piecewise over only the valid query
    ranges of each key chunk; AV emission is skewed one head behind scores
    so exp latency hides under matmuls.
  - Softmax normalization: denominator rows bounce via DRAM broadcast, one
    DVE reciprocal + multiply per head pair (no ScalarE Ln/Exp).
  - out = y @ W_proj via K=128 matmuls; biases applied as DVE adds of
    DMA-broadcast bias tiles (no PE bias matmuls).
"""
import numpy as np

import bass_rust
import concourse.bass as bass
import concourse.mybir as mybir
import concourse.tile as tile
from concourse.bass_utils import run_bass_kernel_spmd
from concourse.masks import make_identity
from concourse.vector_clock import ScopedClock

F32 = mybir.dt.float32
BF16 = mybir.dt.bfloat16
AF = mybir.ActivationFunctionType
ADD = mybir.AluOpType.add

B, T, C, NH, HD = 8, 1024, 768, 12, 64
C3 = 3 * C
SCALE = 1.0 / 8.0  # 1/sqrt(HD)
NEG = -1.0e9  # pre-scale mask addend (exp(SCALE*NEG) == 0)

# ---- score piece layout -------------------------------------------------
# Per head, the 12 valid (kc, ic) S^T tiles split into 128-wide diagonal
# bands plus fully-valid suffixes, packed with zero waste into three
# 1536-col psum groups (3 banks each; no matmul region crosses a bank):
#   Ga: [bands-ic0 0:512 | 384f-ic0 512:896 | 128f-ic0 896:1024 | f1a0 1024:1536]
#   Gb: [f1a1 0:512 | f1a2 512:1024 | f1a3 1024:1536]
#   Gc: [bands-ic1 0:512 | 384f-ic1 512:896 | 128f-ic1 896:1024 |
#        256f-ic0 1024:1280 | 256f-ic1 1280:1536]
# f1aN are the four fully-valid ic1 tiles (kc=N). Slab blocks follow
# Ga,Gb,Gc order. Score piece: (kc, grp, gcol, width, ic, qcol, band).
GW = 1536
BLK = {"Ga": 0, "Gb": 1, "Gc": 2}

# PSUM start=True marks the whole 2KB bank pending-zero; later writes to a
# pending region overwrite (and clear it), others accumulate. So: exactly
# one start=True per bank — its first writer — and start=False elsewhere.
# Score piece: (kc, grp, gcol, width, ic, qcol, start, stop).
STAGE1 = [
    (0, "Ga", 0, 128, 0, 0, True, False),      # bands bank first writer
    (0, "Ga", 512, 384, 0, 128, True, False),  # bank1 first writer
    (0, "Ga", 1024, 512, 1, 0, True, True),    # f1a0: bank2, sole writer
    (1, "Ga", 128, 128, 0, 128, False, False),
    (1, "Gb", 0, 512, 1, 0, True, True),
    (2, "Ga", 256, 128, 0, 256, False, False),
    (2, "Ga", 896, 128, 0, 384, False, True),  # bank1 last
    (2, "Gb", 512, 512, 1, 0, True, True),
    (3, "Ga", 384, 128, 0, 384, False, False),
    (3, "Gb", 1024, 512, 1, 0, True, True),
]  # then mask->Ga (F, T), exp Ga, exp Gb
STAGE2 = [
    (1, "Gc", 1024, 256, 0, 256, True, False),  # bank2 first writer
    (4, "Gc", 0, 128, 1, 0, True, False),       # bands bank first writer
    (4, "Gc", 512, 384, 1, 128, True, False),   # bank1 first writer
    (5, "Gc", 128, 128, 1, 128, False, False),
    (5, "Gc", 1280, 256, 1, 256, False, True),  # bank2 last
    (6, "Gc", 256, 128, 1, 256, False, False),
    (6, "Gc", 896, 128, 1, 384, False, True),   # bank1 last
    (7, "Gc", 384, 128, 1, 384, False, False),
]  # then mask->Gc (F, T), exp Gc

# AV pieces: (kc, grp, gcol, width, outcol, start, stop); the av psum is a
# single bank: one start=True on the first piece, pending-zero handles the
# rest.
AV_IC0 = [
    (0, "Ga", 0, 128, 0, True, False),
    (0, "Ga", 512, 384, 128, False, False),
    (1, "Ga", 128, 128, 128, False, False),
    (2, "Ga", 256, 128, 256, False, False),
    (2, "Ga", 896, 128, 384, False, False),
    (3, "Ga", 384, 128, 384, False, False),
    # the lone Gc-resident piece last, so the AV only stalls on the final
    # exp for its last matmul
    (1, "Gc", 1024, 256, 256, False, True),
]
AV_IC1 = [
    (0, "Ga", 1024, 512, 0, True, False),
    (1, "Gb", 0, 512, 0, False, False),
    (2, "Gb", 512, 512, 0, False, False),
    (3, "Gb", 1024, 512, 0, False, False),
    (4, "Gc", 0, 128, 0, False, False),
    (4, "Gc", 512, 384, 128, False, False),
    (5, "Gc", 128, 128, 128, False, False),
    (5, "Gc", 1280, 256, 256, False, False),
    (6, "Gc", 256, 128, 256, False, False),
    (6, "Gc", 896, 128, 384, False, False),
    (7, "Gc", 384, 128, 384, False, True),
]


class TileContextFixed(tile.TileContext):
    """Splits sem waits beyond walrus's per-instruction cap onto NOPs/Drains."""

    def _split_excess_waits(self, inst):
        si = inst.sync_info
        if si is None or not si.on_wait:
            return []
        cap = 2 if isinstance(inst, mybir.InstEventSemaphore) else 1
        waits = list(si.on_wait)
        if len(waits) <= cap:
            return []
        keep = waits[len(waits) - cap:]
        excess = waits[: len(waits) - cap]
        inst.sync_info = bass_rust.SyncInfo(
            on_wait=keep, on_update=list(si.on_update or [])
        )
        nops = []
        for w in excess:
            nop = mybir.InstNoOp(name=f"I-wsplit-{self.nc.next_id()}")
            nop.engine = inst.engine
            nop.sync_info = bass_rust.SyncInfo(on_wait=[w], on_update=[])
            nops.append(nop)
        return nops

    def _commit_instruction(self, inst, lazy_reg_writes: bool = True):
        for nop in self._split_excess_waits(inst):
            self._add_instruction(nop)
        super()._commit_instruction(inst, lazy_reg_writes)

    def _drain_and_barrier(self, tick_clock, wait_clock):
        drain_inst = self.nc.sync.drain()
        wait_clock.add_sem_waits(
            drain_inst.ins, ScopedClock({None: tick_clock.global_clock})
        )
        si = drain_inst.ins.sync_info
        if si is not None and si.on_wait and len(si.on_wait) > 1:
            waits = list(si.on_wait)
            ups = list(si.on_update) if si.on_update else []
            drain_inst.ins.sync_info = bass_rust.SyncInfo(
                on_wait=[waits[0]], on_update=[]
            )
            for i, w in enumerate(waits[1:]):
                d2 = self.nc.sync.drain()
                d2.ins.sync_info = bass_rust.SyncInfo(
                    on_wait=[w], on_update=ups if i == len(waits) - 2 else []
                )
        self.nc.all_engine_barrier()
        assert self.sems is not None
        popped = self.nc._tile_sem_poison_stack.pop()
        assert popped is self._sem_poison
        self.nc.clear_and_free_semaphores(list(self.sems.allocated().values()))
        self.nc.all_engine_barrier()


def build_nc():
    nc = bass.Bass()
    X = nc.declare_dram_parameter("x", [T, C], F32, isOutput=False)
    WA = nc.declare_dram_parameter("W_attn", [C, C3], F32, isOutput=False)
    BA = nc.declare_dram_parameter("b_attn", [C3], F32, isOutput=False)
    WP = nc.declare_dram_parameter("W_proj", [C, C], F32, isOutput=False)
    BP = nc.declare_dram_parameter("b_proj", [C], F32, isOutput=False)
    OUT = nc.declare_dram_parameter("out", [T, C], F32, isOutput=True)

    with TileContextFixed(nc) as tc:
        with (
            tc.tile_pool(name="const", bufs=1) as const,
            tc.tile_pool(name="dram", bufs=1, space="DRAM") as dram,
        ):
            r_dram = dram.tile([12, T], F32)

            # gpsimd setup ops first so they precede SWDGE desc-gen work
            tri_f = const.tile([128, 128], F32)
            nc.gpsimd.memset(tri_f, 0.0)
            nc.gpsimd.affine_select(
                out=tri_f,
                in_=tri_f,
                compare_op=mybir.AluOpType.is_ge,
                fill=NEG,
                base=0,
                pattern=[[1, 128]],
                channel_multiplier=-1,
            )

            # All front-door loads ride one ordered SP HWDGE stream (DMA
            # transfers serialize on the shared DMA engines, so order IS
            # priority): x0-3, q/k W columns ci-chunk by ci-chunk (qkv
            # matmuls start after the first ci chunk), x4-7 slotted after
            # ci3, then the v slab and biases. W_proj + b_proj ride SWDGE
            # later (phase 2) when the DMA engines are idle.
            xnat_f = []
            xnat_b = []
            for ti in range(8):
                xf = const.tile([128, C], F32, name=f"xf{ti}")
                xnat_f.append(xf)
                xb = const.tile([128, C], BF16, name=f"xb{ti}")
                xnat_b.append(xb)

            def load_x(ti, split=False):
                if split:
                    for h0, h1 in ((0, 384), (384, 768)):
                        nc.sync.dma_start(
                            out=xnat_f[ti][:, h0:h1],
                            in_=X.ap()[128 * ti: 128 * (ti + 1), h0:h1],
                        )
                        nc.vector.tensor_copy(
                            xnat_b[ti][:, h0:h1], xnat_f[ti][:, h0:h1]
                        )
                else:
                    nc.sync.dma_start(
                        out=xnat_f[ti], in_=X.ap()[128 * ti: 128 * (ti + 1), :]
                    )
                    nc.vector.tensor_copy(xnat_b[ti], xnat_f[ti])

            for ti in range(4):
                load_x(ti)
            b_attn_col = const.tile([128, 18], F32)
            nc.sync.dma_start(
                out=b_attn_col, in_=BA.ap().rearrange("(a p) -> p a", p=128)
            )

            ident = const.tile([128, 128], BF16)
            make_identity(nc, ident)
            tri_wide = const.tile([128, 512], BF16)
            for r in range(4):
                nc.vector.tensor_copy(tri_wide[:, 128 * r: 128 * (r + 1)], tri_f)

            w_attn_bf = [
                const.tile([128, C3], BF16, name=f"wab{c}") for c in range(6)
            ]
            w_proj_bf = [
                const.tile([128, C], BF16, name=f"wpb{c}") for c in range(6)
            ]
            vb_f = const.tile([128, C], F32)
            ones_row = const.tile([1, 128], BF16)
            b_proj_bf = const.tile([1, C], BF16)

            with tc.tile_pool(name="wstage", bufs=4) as wst:
                for s in range(3):
                    for c in range(6):
                        stg = wst.tile([128, C], F32, tag="wstage")
                        nc.sync.dma_start(
                            out=stg,
                            in_=WA.ap()[
                                128 * c: 128 * (c + 1), C * s: C * (s + 1)
                            ],
                        )
                        nc.vector.tensor_copy(
                            w_attn_bf[c][:, C * s: C * (s + 1)], stg
                        )
                    if s == 0:
                        for ti in range(4, 8):
                            load_x(ti)
                nc.sync.dma_start(
                    out=vb_f,
                    in_=BA.ap()[2 * C: 3 * C]
                    .rearrange("(a c) -> a c", a=1)
                    .to_broadcast([128, C]),
                )
                # W_proj + b_proj at the tail of the same stream (HWDGE
                # fp32 + DVE cast; SWDGE desc-gen would front-run the queue)
                for c in range(6):
                    stg = wst.tile([128, C], F32, tag="wstage")
                    nc.sync.dma_start(
                        out=stg, in_=WP.ap()[128 * c: 128 * (c + 1), :]
                    )
                    nc.vector.tensor_copy(w_proj_bf[c], stg)
                stg = wst.tile([128, C], F32, tag="wstage")
                nc.sync.dma_start(
                    out=stg[0:1, :], in_=BP.ap().rearrange("(a c) -> a c", a=1)
                )
                nc.vector.tensor_copy(b_proj_bf, stg[0:1, :])

            # ---- persistent activations ----
            xT = [const.tile([128, T], BF16, name=f"xT{c}") for c in range(6)]
            qT = [const.tile([128, T], BF16, name=f"qT{i}") for i in range(6)]
            kT = [const.tile([128, T], BF16, name=f"kT{i}") for i in range(6)]
            v_aug = [
                const.tile([128, NH, HD + 1], BF16, name=f"vau{i}") for i in range(8)
            ]
            y_pair = [const.tile([128, T], BF16, name=f"yp{i}") for i in range(6)]

            # ---- phases 1+2 (shared SBUF pools so pair-0 scores can
            # be emitted during phase 1 and consumed in phase 2) ----
            with (
                tc.tile_pool(name="ptpool", bufs=3) as ptp,
                tc.tile_pool(name="lp", bufs=2) as lp,
            ):
                lrows = {}
                rbs = {}
                rinvs = {}

                def alloc_pair(p):
                    lrows[p] = lp.tile(
                        [128, 2048], F32, tag="lrow", name=f"lr{p}"
                    )
                    rbs[p] = lp.tile([128, T], F32, tag="rb", name=f"rb{p}")
                    rinvs[p] = lp.tile([128, T], F32, tag="rinv", name=f"ri{p}")

                def emit_scores(p, base, slab, pool, split_gc=False):
                    groups = {}

                    def wave(pieces, grp_order, mask_grp):
                        for g in grp_order:
                            if g not in groups:
                                groups[g] = pool.tile(
                                    [128, GW], F32, tag="sc", name=f"sc{g}"
                                )
                        for kc, grp, gcol, w, ic, qc, st, sp in pieces:
                            nc.tensor.matmul(
                                groups[grp][:, gcol: gcol + w],
                                kT[p][base: base + 64, 128 * kc: 128 * (kc + 1)],
                                qT[p][
                                    base: base + 64,
                                    512 * ic + qc: 512 * ic + qc + w,
                                ],
                                start=st,
                                stop=sp,
                                skip_group_check=True,
                            )
                        nc.tensor.matmul(
                            groups[mask_grp][:, 0:512],
                            ident,
                            tri_wide,
                            start=False,
                            stop=True,
                            skip_group_check=True,
                        )
                        for g in grp_order:
                            blk = BLK[g]
                            if split_gc and g == "Gc":
                                # last item: per-bank exps so the final AV's
                                # pieces unblock as each bank lands
                                for b0 in (0, 512, 1024):
                                    nc.scalar.activation(
                                        slab[
                                            :,
                                            GW * blk + b0: GW * blk + b0 + 512,
                                        ],
                                        groups[g][:, b0: b0 + 512],
                                        AF.Exp,
                                        scale=SCALE,
                                    )
                            else:
                                nc.scalar.activation(
                                    slab[:, GW * blk: GW * (blk + 1)],
                                    groups[g],
                                    AF.Exp,
                                    scale=SCALE,
                                )

                    wave(STAGE1, ("Ga", "Gb"), "Ga")
                    wave(STAGE2, ("Gc",), "Gc")

                # ---- phase 1: transposes interleaved with qkv (PE issues
                # in order, so tg0 -> qkv ti=0 -> tg1 -> qkv ti=1 keeps PE
                # fed while the second x half and later W slabs stream) ----
                with tc.tile_pool(name="qkvps", bufs=4, space="PSUM") as qkps:
                    with tc.tile_pool(name="xps", bufs=3, space="PSUM") as xps:

                        def transposes(tg):
                            # x-chunk-major: 6 transposes become ready per
                            # arriving x chunk; two c-columns share one
                            # bank-sized bf16 psum tile.
                            tps = [
                                xps.tile(
                                    [128, 1024], BF16, tag="xps",
                                    name=f"tp{tg}_{c}",
                                )
                                for c in range(3)
                            ]
                            for q in range(4):
                                ti = 4 * tg + q
                                for c in range(6):
                                    nc.tensor.transpose(
                                        tps[c // 2][
                                            :,
                                            512 * (c % 2) + 128 * q: 512 * (c % 2)
                                            + 128 * (q + 1),
                                        ],
                                        xnat_b[ti][:, 128 * c: 128 * (c + 1)],
                                        ident,
                                    )
                            for c in range(6):
                                src = tps[c // 2][
                                    :, 512 * (c % 2): 512 * (c % 2 + 1)
                                ]
                                if (c + tg) % 2:
                                    nc.scalar.copy(
                                        xT[c][:, 512 * tg: 512 * (tg + 1)], src
                                    )
                                else:
                                    nc.vector.tensor_copy(
                                        xT[c][:, 512 * tg: 512 * (tg + 1)], src
                                    )

                        def qkv_part(ti, ci0):
                            # c-major over blocks of 3 columns: each arriving
                            # W chunk unlocks matmuls immediately
                            for blk in (0, 3):
                                pss = [
                                    qkps.tile(
                                        [128, 512], F32, tag="qkps",
                                        name=f"qp{j}",
                                    )
                                    for j in range(3)
                                ]
                                for c in range(6):
                                    for j in range(3):
                                        ci = ci0 + blk + j
                                        nc.tensor.matmul(
                                            pss[j],
                                            w_attn_bf[c][
                                                :, 128 * ci: 128 * (ci + 1)
                                            ],
                                            xT[c][:, 512 * ti: 512 * (ti + 1)],
                                            start=(c == 0),
                                            stop=(c == 5),
                                        )
                                for j in range(3):
                                    ci = ci0 + blk + j
                                    dst = qT[ci] if ci < 6 else kT[ci - 6]
                                    nc.scalar.activation(
                                        dst[:, 512 * ti: 512 * (ti + 1)],
                                        pss[j],
                                        AF.Identity,
                                        bias=b_attn_col[:, ci: ci + 1],
                                    )

                        transposes(0)
                        qkv_part(0, 0)
                        transposes(1)
                        qkv_part(1, 0)
                        qkv_part(0, 6)
                        qkv_part(1, 6)

                    # pair-0 head-0 scores now, one psum group at a time
                    # (single 3-bank slot), interleaved with v tiles so each
                    # group's exp hides under the next v matmuls and phase 2
                    # starts with a warm pipeline
                    alloc_pair(0)
                    slab0 = ptp.tile([128, 3 * GW], BF16, tag="slab", name="sl00")

                    def v_tile(ti):
                        for ni, (n0, n) in enumerate(((0, 512), (512, 256))):
                            ps = qkps.tile([128, 512], F32, tag="qkps")
                            for c in range(6):
                                nc.tensor.matmul(
                                    ps[:, :n],
                                    xT[c][:, 128 * ti: 128 * (ti + 1)],
                                    w_attn_bf[c][:, 2 * C + n0: 2 * C + n0 + n],
                                    start=(c == 0),
                                    stop=(c == 5),
                                )
                            hn = n // HD
                            nc.vector.tensor_tensor(
                                v_aug[ti][:, 8 * ni: 8 * ni + hn, 0:HD],
                                ps[:, :n].rearrange("p (h d) -> p h d", d=HD),
                                vb_f[:, n0: n0 + n].rearrange(
                                    "p (h d) -> p h d", d=HD
                                ),
                                ADD,
                            )
                        nc.vector.memset(v_aug[ti][:, :, HD: HD + 1], 1.0)

                    S1A = [pc for pc in STAGE1 if pc[1] == "Ga"]
                    S1B = [pc for pc in STAGE1 if pc[1] == "Gb"]

                    def score_group(pool, slab, grp, pieces, masked):
                        g = pool.tile([128, GW], F32, tag="sc", name=f"s0{grp}")
                        for kc, _, gcol, w, ic, qc, st, sp in pieces:
                            nc.tensor.matmul(
                                g[:, gcol: gcol + w],
                                kT[0][0:64, 128 * kc: 128 * (kc + 1)],
                                qT[0][0:64, 512 * ic + qc: 512 * ic + qc + w],
                                start=st,
                                stop=sp,
                                skip_group_check=True,
                            )
                        if masked:
                            nc.tensor.matmul(
                                g[:, 0:512],
                                ident,
                                tri_wide,
                                start=False,
                                stop=True,
                                skip_group_check=True,
                            )
                        blk = BLK[grp]
                        nc.scalar.activation(
                            slab[:, GW * blk: GW * (blk + 1)],
                            g,
                            AF.Exp,
                            scale=SCALE,
                        )

                    with tc.tile_pool(name="sc0", bufs=1, space="PSUM") as sc0:
                        score_group(sc0, slab0, "Ga", S1A, True)
                        v_tile(0)
                        score_group(sc0, slab0, "Gb", S1B, False)
                        v_tile(1)
                        score_group(sc0, slab0, "Gc", STAGE2, True)
                        for ti in range(2, 8):
                            v_tile(ti)

                # ---- phase 2: attention, one item per head, AV skewed ----
                with (
                    tc.tile_pool(name="spool", bufs=2, space="PSUM") as sps,
                    tc.tile_pool(name="avpool", bufs=2, space="PSUM") as avps,
                ):
                    nc.vector.memset(ones_row, 1.0)

                    def emit_av(p, bi, base, slab, ic, pieces):
                        h = 2 * p + bi
                        ps = avps.tile([65, 512], F32, tag="av")
                        for kc, grp, gcol, w, oc, st, sp in pieces:
                            col = GW * BLK[grp] + gcol
                            nc.tensor.matmul(
                                ps[:, oc: oc + w],
                                v_aug[kc][:, h, :],
                                slab[:, col: col + w],
                                start=st,
                                stop=sp,
                                skip_group_check=True,
                            )
                        nc.vector.tensor_copy(
                            y_pair[p][base: base + 64, 512 * ic: 512 * (ic + 1)],
                            ps[0:64, :],
                        )
                        nc.vector.tensor_copy(
                            lrows[p][
                                64:65,
                                1024 * bi + 512 * ic: 1024 * bi + 512 * (ic + 1),
                            ],
                            ps[64:65, :],
                        )
                        if ic == 1:
                            # bounce this base's denominator row immediately
                            nc.sync.dma_start(
                                out=r_dram[2 * p + bi: 2 * p + bi + 1, :],
                                in_=lrows[p][64:65, 1024 * bi: 1024 * (bi + 1)],
                            )
                            nc.sync.dma_start(
                                out=rbs[p][64 * bi: 64 * bi + 64, :],
                                in_=r_dram[
                                    2 * p + bi: 2 * p + bi + 1, :
                                ].to_broadcast([64, T]),
                            )
                            if bi == 1 and p < 5:
                                finalize_pair(p)

                    def half_finalize(p, ic):
                        sl = slice(512 * ic, 512 * (ic + 1))
                        nc.vector.reciprocal(rinvs[p][:, sl], rbs[p][:, sl])
                        nc.vector.tensor_mul(
                            y_pair[p][:, sl], y_pair[p][:, sl], rinvs[p][:, sl]
                        )

                    def finalize_pair(p):
                        nc.vector.reciprocal(rinvs[p], rbs[p])
                        nc.vector.tensor_mul(y_pair[p], y_pair[p], rinvs[p])
                        lrows.pop(p)
                        rbs.pop(p)
                        rinvs.pop(p)

                    items = [(p, bi) for p in range(6) for bi in (0, 1)]
                    prev = None
                    for p, bi in items:
                        if bi == 0 and p > 0:
                            alloc_pair(p)
                        base = 64 * bi
                        if p == 0 and bi == 0:
                            slab = slab0
                        else:
                            slab = ptp.tile(
                                [128, 3 * GW], BF16, tag="slab", name=f"sl{p}_{bi}"
                            )
                            emit_scores(
                                p, base, slab, sps, split_gc=(p == 5 and bi == 1)
                            )
                        if prev is not None:
                            emit_av(*prev, 1, AV_IC1)
                        emit_av(p, bi, base, slab, 0, AV_IC0)
                        prev = (p, bi, base, slab)
                    emit_av(*prev, 1, AV_IC1)
                    # last pair: normalize the early-bounced ic0 half while
                    # the ic1 broadcast is still in flight
                    half_finalize(5, 0)
                    half_finalize(5, 1)
                    lrows.pop(5)
                    rbs.pop(5)
                    rinvs.pop(5)

            # ---- phase 3: output projection ----
            # First 4 ti accumulate ci=0..4 into held psum while the last
            # pair's normalization (DMA bounce + reciprocal) completes, then
            # ci=5 finishes each. Tile is [128,1024] (2 banks) so both the
            # 512- and 256-wide regions stay bank-aligned.
            with (
                tc.tile_pool(name="pps", bufs=4, space="PSUM") as pps,
                tc.tile_pool(name="ops", bufs=3) as ops,
            ):
                held = []
                for ti in range(4):
                    psa = pps.tile([128, 512], F32, tag="pp", name=f"pa{ti}")
                    psb = pps.tile([128, 512], F32, tag="pq", name=f"pb{ti}")
                    held.append((psa, psb))
                # ci-major so each pair's normalization unblocks a block of
                # matmuls as it lands
                for ci in range(5):
                    for ti in range(4):
                        psa, psb = held[ti]
                        for ps, n0, n in ((psa, 0, 512), (psb, 512, 256)):
                            nc.tensor.matmul(
                                ps[:, :n],
                                y_pair[ci][:, 128 * ti: 128 * (ti + 1)],
                                w_proj_bf[ci][:, n0: n0 + n],
                                start=(ci == 0),
                                stop=False,
                                skip_group_check=True,
                            )
                for ti in range(8):
                    if ti < 4:
                        psa, psb = held[ti]
                    else:
                        psa = pps.tile([128, 512], F32, tag="pp", name=f"pa{ti}")
                        psb = pps.tile([128, 512], F32, tag="pq", name=f"pb{ti}")
                    osb = ops.tile([128, C], F32, tag="osb")
                    for ps, n0, n in ((psa, 0, 512), (psb, 512, 256)):
                        cis = (5,) if ti < 4 else range(6)
                        for ci in cis:
                            nc.tensor.matmul(
                                ps[:, :n],
                                y_pair[ci][:, 128 * ti: 128 * (ti + 1)],
                                w_proj_bf[ci][:, n0: n0 + n],
                                start=(ci == 0),
                                stop=False,
                                skip_group_check=True,
                            )
                        # bias via K=1 ones matmul; psum->sbuf copy on the
                        # otherwise-idle ScalarE (keeps DVE off the tail)
                        nc.tensor.matmul(
                            ps[:, :n],
                            ones_row,
                            b_proj_bf[:, n0: n0 + n],
                            start=False,
                            stop=True,
                            skip_group_check=True,
                        )
                        nc.scalar.copy(osb[:, n0: n0 + n], ps[:, :n])
                        nc.sync.dma_start(
                            out=OUT.ap()[128 * ti: 128 * (ti + 1), n0: n0 + n],
                            in_=osb[:, n0: n0 + n],
                        )

    return nc


_NC = None
LAST_EXEC_NS = None


def _get_nc():
    global _NC
    if _NC is None:
        _NC = build_nc()
    return _NC


def kernel(x, W_attn, b_attn, W_proj, b_proj):
    x = np.ascontiguousarray(np.asarray(x, dtype=np.float32))
    W_attn = np.ascontiguousarray(np.asarray(W_attn, dtype=np.float32))
    b_attn = np.ascontiguousarray(np.asarray(b_attn, dtype=np.float32))
    W_proj = np.ascontiguousarray(np.asarray(W_proj, dtype=np.float32))
    b_proj = np.ascontiguousarray(np.asarray(b_proj, dtype=np.float32))

    nc = _get_nc()
    in_maps = [
        {
            "x": x[b],
            "W_attn": W_attn,
            "b_attn": b_attn,
            "W_proj": W_proj,
            "b_proj": b_proj,
        }
        for b in range(B)
    ]
    res = run_bass_kernel_spmd(nc, in_maps, core_ids=list(range(B)))
    global LAST_EXEC_NS
    if res.exec_time_ns is not None:
        LAST_EXEC_NS = res.exec_time_ns
    return np.stack([r["out"] for r in res.results], axis=0)


if __name__ == "__main__":
    rng = np.random.default_rng(0)
    inputs = {
        "x": rng.standard_normal((B, T, C), dtype=np.float32),
        "W_attn": (rng.standard_normal((C, C3), dtype=np.float32) * 0.02),
        "b_attn": np.zeros((C3,), np.float32),
        "W_proj": (rng.standard_normal((C, C), dtype=np.float32) * 0.02),
        "b_proj": np.zeros((C,), np.float32),
    }
    out = kernel(**inputs)
    print("out shape", out.shape, out.dtype)
